# revision 11
# baseline (speedup 1.0000x reference)
"""Trainium2 kernel for out[b] = P @ X[b] @ P.T  (basis-change of a density matrix).

P (7140, 1024) is a 0/1 matrix with exactly one 1 per column, so the op is a
pure scatter: out[b][rowmap[i], rowmap[j]] = X[b][i, j], zeros elsewhere.

Sharding: 8 cores, core c owns batch b=c//4 and input rows [256*q, 256*q+256)
(q=c%4), i.e. 256 mapped output rows plus a 1529-row chunk of the all-zero
output rows. Every core runs the identical program (SPMD) on its input slice:
  - DMA-load its (256, 1024) X slice into SBUF (rows on partitions, 2 per
    partition),
  - scatter-copy the 4-wide column blocks into a wide W tile covering the
    nonzero column span [420, 5994); the column map is the same fixed
    permutation for every row. Copies are split between the DVE and ACT
    engines by column region (DVE memsets both regions, each engine fills
    its own),
  - DMA-store the 256 wide rows to its output shard.
Unwritten shard rows/columns are zero by the ExternalOutput contract
(run_bass_kernel_spmd pre-zeros / donates zero-initialized output buffers).
The host then unshards by placing each core's rows at their output positions.
"""

import os
import numpy as np

_CORES = 8
_B = 2
_DIN = 1024
_DOUT = 7140
_RPC = _DIN // 4                          # 256 mapped rows per core
_ZPC = (_DOUT - _DIN) * _B // _CORES      # 1529 zero rows per core
_SHARD_ROWS = _RPC + _ZPC                 # 1785


def _rowmap_from_P(P):
    """Output row index for each input row/col: the row of the single 1 in
    each column of P."""
    return np.argmax(np.asarray(P), axis=0).astype(np.int64)


def _runs(rowmap):
    """Maximal runs where both src and dst advance by 1: list of
    (src_start, dst_start, length)."""
    runs = []
    s = 0
    n = len(rowmap)
    for i in range(1, n + 1):
        if i == n or rowmap[i] != rowmap[i - 1] + 1:
            runs.append((s, int(rowmap[s]), i - s))
            s = i
    return runs


def _split_runs(runs, lo, width):
    """Pick k so ACT (runs[:k] + its region memset... memset is on DVE) and
    DVE (runs[k:] + both memsets) finish at about the same time, using the
    errata cycle formulas: DVE op ~ (58 + FD/2)/0.96 ns,
    ACT op ~ (224 + FD/2)/1.2 ns (FD = 2*len)."""

    def dve_ns(fd):
        return (58 + fd / 2) / 0.96

    def act_ns(fd):
        return (224 + fd / 2) / 1.2

    best_k, best_t = 0, None
    for k in range(len(runs) + 1):
        b = (runs[k][1] - lo) if k < len(runs) else width
        t_dve = dve_ns(2 * width) + sum(dve_ns(2 * r[2]) for r in runs[k:])
        t_act = dve_ns(2 * b) + sum(act_ns(2 * r[2]) for r in runs[:k])
        t = max(t_act, t_dve)
        if best_t is None or t < best_t:
            best_k, best_t = k, t
    return best_k


def _build_program(runs, lo, hi, iters=1):
    import concourse.bass as bass
    import concourse.mybir as mybir

    f32 = mybir.dt.float32
    width = hi - lo
    nc = bass.Bass()
    x = nc.dram_tensor("x", [_RPC, _DIN], f32, kind="ExternalInput")
    out = nc.dram_tensor("out", [_SHARD_ROWS, _DOUT], f32, kind="ExternalOutput")

    k = _split_runs(runs, lo, width)
    bcol = (runs[k][1] - lo) if k < len(runs) else width
    act_runs = runs[:k]
    dve_runs = runs[k:]
    dpi = 16 * (1 + 2 * int(bcol > 0) + 2 * int(bcol < width))  # dma incs/iter

    with (
        nc.sbuf_tensor([128, 2, width], f32) as W,
        nc.sbuf_tensor([128, 2, _DIN], f32) as X,
        nc.semaphore("dma_sem") as dma_sem,
        nc.semaphore("dve_sem") as dve_sem,
        nc.semaphore("act_sem") as act_sem,
        nc.semaphore("ms_sem") as ms_sem,
        nc.Block() as block,
    ):
        xr = x.rearrange("(t p) c -> p t c", t=2)

        @block.sync
        def _(sync):
            for i in range(iters):
                if i > 0:
                    # X is about to be overwritten: iter i-1 copies must be done.
                    if dve_runs:
                        sync.wait_ge(dve_sem, i)
                    if act_runs:
                        sync.wait_ge(act_sem, i)
                sync.dma_start(out=X[:], in_=xr).then_inc(dma_sem, 16)
                if bcol > 0:
                    sync.wait_ge(act_sem if act_runs else dve_sem, i + 1)
                    for t in range(2):
                        sync.dma_start(
                            out=out[128 * t : 128 * (t + 1), lo : lo + bcol],
                            in_=W[:, t, 0:bcol],
                        ).then_inc(dma_sem, 16)
                if bcol < width:
                    sync.wait_ge(dve_sem, i + 1)
                    for t in range(2):
                        sync.dma_start(
                            out=out[128 * t : 128 * (t + 1), lo + bcol : hi],
                            in_=W[:, t, bcol:width],
                        ).then_inc(dma_sem, 16)
            sync.wait_ge(dma_sem, dpi * iters)

        @block.vector
        def _(vector):
            for i in range(iters):
                if i > 0:
                    # W is about to be overwritten: iter i-1 stores must be done.
                    vector.wait_ge(dma_sem, dpi * i)
                if bcol > 0:
                    vector.memset(W[:, :, 0:bcol], 0.0).then_inc(ms_sem, 1)
                if bcol < width:
                    vector.memset(W[:, :, bcol:width], 0.0)
                vector.wait_ge(dma_sem, dpi * i + 16)
                for j, (src, dst, ln) in enumerate(dve_runs):
                    ins = vector.tensor_copy(
                        W[:, :, dst - lo : dst - lo + ln], X[:, :, src : src + ln]
                    )
                    if j == len(dve_runs) - 1:
                        ins.then_inc(dve_sem, 1)

        @block.scalar
        def _(scalar):
            for i in range(iters):
                if act_runs:
                    scalar.wait_ge(ms_sem, i + 1)
                    scalar.wait_ge(dma_sem, dpi * i + 16)
                    for j, (src, dst, ln) in enumerate(act_runs):
                        ins = scalar.copy(
                            W[:, :, dst - lo : dst - lo + ln],
                            X[:, :, src : src + ln],
                        )
                        if j == len(act_runs) - 1:
                            ins.then_inc(act_sem, 1)

    return nc


def _shard_inputs(input_state):
    in_maps = []
    for c in range(_CORES):
        b, q = divmod(c, 4)
        sl = np.ascontiguousarray(
            input_state[b, _RPC * q : _RPC * (q + 1), :], dtype=np.float32
        )
        in_maps.append({"x": sl})
    return in_maps


def _unshard(results, rowmap):
    unmapped = np.setdiff1d(np.arange(_DOUT), rowmap)
    out = np.empty((_B, _DOUT, _DOUT), np.float32)
    for c in range(_CORES):
        b, q = divmod(c, 4)
        shard = results[c]["out"]
        out[b, rowmap[_RPC * q : _RPC * (q + 1)], :] = shard[:_RPC]
        out[b, unmapped[_ZPC * q : _ZPC * (q + 1)], :] = shard[_RPC:]
    return out


def kernel(input_state, P):
    from concourse.bass_utils import run_bass_kernel_spmd

    input_state = np.asarray(input_state)
    rowmap = _rowmap_from_P(P)
    runs = _runs(rowmap)
    nc = _build_program(runs, int(rowmap.min()), int(rowmap.max()) + 1)
    res = run_bass_kernel_spmd(
        nc, _shard_inputs(input_state), core_ids=list(range(_CORES)), trace=False
    )
    return _unshard(res.results, rowmap)


# revision 24
# speedup vs baseline: 1.4077x; 1.4077x over previous
"""Trainium2 kernel for out[b] = P @ X[b] @ P.T  (basis-change of a density matrix).

P (7140, 1024) is a 0/1 matrix with exactly one 1 per column, so the op is a
pure scatter: out[b][rowmap[i], rowmap[j]] = X[b][i, j], zeros elsewhere.

Structure of the map (derived from P at runtime, asserted): input columns
group into 16 "lines" of 64; line l lands in the output as a 169-wide
cluster at base(l) (quadratic spacing), and within every cluster the 64
values sit at the same 15 runs of offsets. This lets the kernel:
  - keep a *packed* W tile [128 part, 2 row-groups, 16*169] in SBUF where the
    line dimension has constant stride 169,
  - perform the whole column scatter with ~15 wide DVE copies per line-half
    (access patterns may use different strides on input and output, so one op
    covers all lines x both row-groups x 4 channels),
  - let the 16 per-cluster store DMAs do the quadratic base(l) placement for
    free (DMA is bandwidth-bound, not op-bound).

Sharding: 8 cores, core c owns batch b=c//4 and input rows [256*q, 256*q+256)
(q=c%4), i.e. 256 mapped output rows plus a 1529-row chunk of the all-zero
output rows. Every core runs the identical program (SPMD) on its input slice.
Unwritten shard rows/columns are zero by the ExternalOutput contract
(run_bass_kernel_spmd pre-zeros / donates zero-initialized output buffers).
The host then unshards by placing each core's rows at their output positions.

Per-core program: GPSIMD memsets W (hidden under the X load), DVE does the
scatter copies (two line-halves so stores pipeline), SP issues loads/stores.
"""

import os
import numpy as np

_CORES = 8
_B = 2
_DIN = 1024
_DOUT = 7140
_RPC = _DIN // 4                          # 256 mapped rows per core
_ZPC = (_DOUT - _DIN) * _B // _CORES      # 1529 zero rows per core
_SHARD_ROWS = _RPC + _ZPC                 # 1785
_NL = 16                                  # lines
_LW = _DIN // _NL                         # 64 input cols per line


def _rowmap_from_P(P):
    """Output row index for each input row/col: the row of the single 1 in
    each column of P."""
    return np.argmax(np.asarray(P), axis=0).astype(np.int64)


def _cluster_structure(rowmap):
    """Split the map into 16 uniform line-clusters.

    Returns (bases, cw, runs) where bases[l] is the output column of cluster
    l, cw the common cluster width, and runs the list of
    (src_off, dst_off, length) copy runs shared by every cluster."""
    rm = rowmap.reshape(_NL, _LW)
    bases = rm[:, 0].copy()
    offs = rm - bases[:, None]
    if not (offs == offs[0]).all():
        raise ValueError("P does not have the expected uniform line structure")
    off0 = offs[0]
    if not ((np.diff(off0) >= 1).all() and off0[0] == 0):
        raise ValueError("cluster offsets not monotonic")
    cw = int(off0[-1]) + 1
    b = np.sort(bases)
    if (np.diff(b) < cw).any() or b[-1] + cw > _DOUT:
        raise ValueError("clusters overlap")
    runs = []
    s = 0
    for i in range(1, _LW + 1):
        if i == _LW or off0[i] != off0[i - 1] + 1:
            runs.append((s, int(off0[s]), i - s))
            s = i
    return [int(v) for v in bases], cw, runs


def _build_program(bases, cw, runs, iters=1, nloads=2):
    import concourse.bass as bass
    import concourse.mybir as mybir

    f32 = mybir.dt.float32
    nc = bass.Bass()
    x = nc.dram_tensor("x", [_RPC, _DIN], f32, kind="ExternalInput")
    out = nc.dram_tensor("out", [_SHARD_ROWS, _DOUT], f32, kind="ExternalOutput")

    qw = _NL * cw // 4       # W cols per zero-quarter (4 lines)
    dpi = 16 * _NL           # dma_sem incs per iter (16 stores)
    nb = min(2, iters)       # W/X buffers (ping-pong across iterations)
    lw = _DIN // nloads      # x cols per load

    # store ownership: half0 = clusters 0-7, half1 = 8-15. SP and ACT are
    # HWDGE issuers, Pool is SWDGE (runs when DVE is idle; shares an SBUF
    # port with DVE).
    sp_h0, sp_h1 = [0, 1, 2, 3], [8, 9]
    act_h0, act_h1 = [4, 5, 6, 7], [10]
    pool_h1 = [11, 12, 13, 14, 15]

    from contextlib import ExitStack

    with ExitStack() as ctx:
        Ws = [
            ctx.enter_context(nc.sbuf_tensor(f"Wbuf{j}", [128, 2, _NL * cw], f32))
            for j in range(nb)
        ]
        Xs = [
            ctx.enter_context(nc.sbuf_tensor(f"Xbuf{j}", [128, 2, _DIN], f32))
            for j in range(nb)
        ]
        dma_sem = ctx.enter_context(nc.semaphore("dma_sem"))
        dve_sem = ctx.enter_context(nc.semaphore("dve_sem"))
        ms_sem = ctx.enter_context(nc.semaphore("ms_sem"))
        l_sems = [
            [
                ctx.enter_context(nc.semaphore(f"l{j}_{g}_sem"))
                for g in range(nloads)
            ]
            for j in range(nb)
        ]
        block = ctx.enter_context(nc.Block())

        # 4D views: [partition, row-group t, line l, within-line col]
        Wvs = [W[:].rearrange("p t (l w) -> p t l w", w=cw) for W in Ws]
        Xvs = [X[:].rearrange("p t (l w) -> p t l w", w=_LW) for X in Xs]

        def store_cluster(eng, j, l):
            return eng.dma_start(
                out=out[0:_RPC, bases[l] : bases[l] + cw].rearrange(
                    "(t p) c -> p t c", t=2
                ),
                in_=Ws[j][:, :, l * cw : (l + 1) * cw],
            ).then_inc(dma_sem, 16)

        def issue_loads(eng, i):
            # Load iteration i's X data into buffer i%nb.
            j = i % nb
            for g in range(nloads):
                eng.dma_start(
                    out=Xs[j][:, :, lw * g : lw * (g + 1)],
                    in_=x[:, lw * g : lw * (g + 1)].rearrange(
                        "(t p) c -> p t c", t=2
                    ),
                ).then_inc(l_sems[j][g], 16)

        # One-time zero fill of the gap columns in both W buffers (Pool).
        # The 15 data runs are overwritten by DVE every iteration; the gap
        # columns are never written again, so they stay zero.
        ms_at = {}
        ms_order = []
        for half in range(2):
            for j in range(nb):
                for q in (2 * half, 2 * half + 1):
                    ms_order.append((j, q))
        for idx, (j, q) in enumerate(ms_order):
            ms_at[(j, q)] = idx + 1

        @block.gpsimd
        def _(gpsimd):
            for j, q in ms_order:
                gpsimd.memset(Ws[j][:, :, qw * q : qw * (q + 1)], 0.0).then_inc(
                    ms_sem, 1
                )
            for i in range(iters):
                j = i % nb
                gpsimd.wait_ge(dve_sem, 2 * i + 2)
                for l in pool_h1:
                    store_cluster(gpsimd, j, l)

        @block.sync
        def _(sync):
            # Bootstrap loads for the first two iterations, then prefetch
            # loads for iter i+2 inside iter i (guarded by dve_sem >= 2i+2:
            # copies of iter i, the last reader of that X buffer, are done).
            issue_loads(sync, 0)
            if iters > 1:
                issue_loads(sync, 1)
            for i in range(iters):
                j = i % nb
                sync.wait_ge(dve_sem, 2 * i + 1)
                for l in sp_h0:
                    store_cluster(sync, j, l)
                sync.wait_ge(dve_sem, 2 * i + 2)
                if i + 2 < iters:
                    issue_loads(sync, i + 2)
                for l in sp_h1:
                    store_cluster(sync, j, l)
            sync.wait_ge(dma_sem, dpi * iters)

        @block.scalar
        def _(scalar):
            for i in range(iters):
                j = i % nb
                scalar.wait_ge(dve_sem, 2 * i + 1)
                for l in act_h0:
                    store_cluster(scalar, j, l)
                scalar.wait_ge(dve_sem, 2 * i + 2)
                for l in act_h1:
                    store_cluster(scalar, j, l)

        @block.vector
        def _(vector):
            for i in range(iters):
                j = i % nb
                if i >= nb:
                    # W[j]'s data runs about to be overwritten: stores of
                    # iter i-nb must be done.
                    vector.wait_ge(dma_sem, dpi * (i - nb + 1))
                uses = i // nb + 1
                for h in range(2):
                    if i < nb:
                        # first use of this buffer: its gap zeros must exist
                        vector.wait_ge(ms_sem, ms_at[(j, 2 * h + 1)])
                    for g in range(nloads):
                        if (g * lw < 512 * (h + 1)) and (lw * (g + 1) > 512 * h):
                            vector.wait_ge(l_sems[j][g], 16 * uses)
                    lsl = slice(8 * h, 8 * (h + 1))
                    for jj, (src, dst, ln) in enumerate(runs):
                        ins = vector.tensor_copy(
                            Wvs[j][:, :, lsl, dst : dst + ln],
                            Xvs[j][:, :, lsl, src : src + ln],
                        )
                        if jj == len(runs) - 1:
                            ins.then_inc(dve_sem, 1)

    return nc


def _shard_inputs(input_state):
    in_maps = []
    for c in range(_CORES):
        b, q = divmod(c, 4)
        sl = np.ascontiguousarray(
            input_state[b, _RPC * q : _RPC * (q + 1), :], dtype=np.float32
        )
        in_maps.append({"x": sl})
    return in_maps


def _unshard(results, rowmap):
    unmapped = np.setdiff1d(np.arange(_DOUT), rowmap)
    out = np.empty((_B, _DOUT, _DOUT), np.float32)
    for c in range(_CORES):
        b, q = divmod(c, 4)
        shard = results[c]["out"]
        out[b, rowmap[_RPC * q : _RPC * (q + 1)], :] = shard[:_RPC]
        out[b, unmapped[_ZPC * q : _ZPC * (q + 1)], :] = shard[_RPC:]
    return out


def kernel(input_state, P):
    from concourse.bass_utils import run_bass_kernel_spmd

    input_state = np.asarray(input_state)
    rowmap = _rowmap_from_P(P)
    bases, cw, runs = _cluster_structure(rowmap)
    nc = _build_program(bases, cw, runs)
    res = run_bass_kernel_spmd(
        nc, _shard_inputs(input_state), core_ids=list(range(_CORES)), trace=False
    )
    return _unshard(res.results, rowmap)


# revision 26
# speedup vs baseline: 1.5364x; 1.0914x over previous
"""Trainium2 kernel for out[b] = P @ X[b] @ P.T  (basis-change of a density matrix).

P (7140, 1024) is a 0/1 matrix with exactly one 1 per column, so the op is a
pure scatter: out[b][rowmap[i], rowmap[j]] = X[b][i, j], zeros elsewhere.

Structure of the map (derived from P at runtime, asserted): input columns
group into 16 "lines" of 64; line l lands in the output as a 169-wide
cluster at base(l) (quadratic spacing), and within every cluster the 64
values sit at the same 15 runs of offsets. This lets the kernel:
  - keep a *packed* W tile [128 part, 2 row-groups, 16*169] in SBUF where the
    line dimension has constant stride 169,
  - perform the whole column scatter with ~15 wide DVE copies per line-half
    (access patterns may use different strides on input and output, so one op
    covers all lines x both row-groups x 4 channels),
  - let the 16 per-cluster store DMAs do the quadratic base(l) placement for
    free (DMA is bandwidth-bound, not op-bound).

Sharding: 8 cores, core c owns batch b=c//4 and input rows [256*q, 256*q+256)
(q=c%4), i.e. 256 mapped output rows plus a 1529-row chunk of the all-zero
output rows. Every core runs the identical program (SPMD) on its input slice.
Unwritten shard rows/columns are zero by the ExternalOutput contract
(run_bass_kernel_spmd pre-zeros / donates zero-initialized output buffers).
The host then unshards by placing each core's rows at their output positions.

Per-core program: GPSIMD memsets W (hidden under the X load), DVE does the
scatter copies (two line-halves so stores pipeline), SP issues loads/stores.
"""

import os
import numpy as np

_CORES = 8
_B = 2
_DIN = 1024
_DOUT = 7140
_RPC = _DIN // 4                          # 256 mapped rows per core
_ZPC = (_DOUT - _DIN) * _B // _CORES      # 1529 zero rows per core
_SHARD_ROWS = _RPC + _ZPC                 # 1785
_NL = 16                                  # lines
_LW = _DIN // _NL                         # 64 input cols per line


def _rowmap_from_P(P):
    """Output row index for each input row/col: the row of the single 1 in
    each column of P."""
    return np.argmax(np.asarray(P), axis=0).astype(np.int64)


def _cluster_structure(rowmap):
    """Split the map into 16 uniform line-clusters.

    Returns (bases, cw, runs) where bases[l] is the output column of cluster
    l, cw the common cluster width, and runs the list of
    (src_off, dst_off, length) copy runs shared by every cluster."""
    rm = rowmap.reshape(_NL, _LW)
    bases = rm[:, 0].copy()
    offs = rm - bases[:, None]
    if not (offs == offs[0]).all():
        raise ValueError("P does not have the expected uniform line structure")
    off0 = offs[0]
    if not ((np.diff(off0) >= 1).all() and off0[0] == 0):
        raise ValueError("cluster offsets not monotonic")
    cw = int(off0[-1]) + 1
    b = np.sort(bases)
    if (np.diff(b) < cw).any() or b[-1] + cw > _DOUT:
        raise ValueError("clusters overlap")
    runs = []
    s = 0
    for i in range(1, _LW + 1):
        if i == _LW or off0[i] != off0[i - 1] + 1:
            runs.append((s, int(off0[s]), i - s))
            s = i
    return [int(v) for v in bases], cw, runs


def _build_program(bases, cw, runs, iters=1):
    import concourse.bass as bass
    import concourse.mybir as mybir

    f32 = mybir.dt.float32
    nc = bass.Bass()
    x = nc.dram_tensor("x", [_RPC, _DIN], f32, kind="ExternalInput")
    out = nc.dram_tensor("out", [_SHARD_ROWS, _DOUT], f32, kind="ExternalOutput")

    # Pair layout: pair m = lines (2m, 2m+1), stored as ONE DMA spanning
    # [base(2m), base(2m+1)+cw) — the inter-cluster gap is materialized as
    # zeros in W (bigger descriptors beat fewer bytes on HW). Within W, pair
    # m is packed at woff[m]; line 2m at woff[m], line 2m+1 at woff[m]+s[m].
    npair = _NL // 2
    s = [bases[2 * m + 1] - bases[2 * m] for m in range(npair)]
    span = [s[m] + cw for m in range(npair)]
    woff = [0] * npair
    for m in range(1, npair):
        woff[m] = woff[m - 1] + span[m - 1]
    wtot = woff[-1] + span[-1]

    dpi = 16 * npair         # dma_sem incs per iter (8 pair stores)
    nb = min(2, iters)       # W/X buffers (ping-pong across iterations)

    # store ownership: SP pairs 0-3 (half0), ACT pairs 4-5, Pool pairs 6-7
    sp_pairs, act_pairs, pool_pairs = [0, 1, 2, 3], [4, 5], [6, 7]

    from contextlib import ExitStack

    with ExitStack() as ctx:
        Ws = [
            ctx.enter_context(nc.sbuf_tensor(f"Wbuf{j}", [128, 2, wtot], f32))
            for j in range(nb)
        ]
        Xs = [
            ctx.enter_context(nc.sbuf_tensor(f"Xbuf{j}", [128, 2, _DIN], f32))
            for j in range(nb)
        ]
        dma_sem = ctx.enter_context(nc.semaphore("dma_sem"))
        dve_sem = ctx.enter_context(nc.semaphore("dve_sem"))
        ms_sem = ctx.enter_context(nc.semaphore("ms_sem"))
        l_sems = [ctx.enter_context(nc.semaphore(f"l{j}_sem")) for j in range(nb)]
        block = ctx.enter_context(nc.Block())

        def store_pair(eng, j, m):
            return eng.dma_start(
                out=out[0:_RPC, bases[2 * m] : bases[2 * m] + span[m]].rearrange(
                    "(t p) c -> p t c", t=2
                ),
                in_=Ws[j][:, :, woff[m] : woff[m] + span[m]],
            ).then_inc(dma_sem, 16)

        def issue_load(eng, i):
            j = i % nb
            eng.dma_start(
                out=Xs[j][:],
                in_=x.rearrange("(t p) c -> p t c", t=2),
            ).then_inc(l_sems[j], 16)

        # One-time zero fill (Pool memsets). Data runs are overwritten by
        # DVE every iteration; gap columns are never written again.
        # Quarter q of buffer j covers pairs (2q, 2q+1).
        ms_at = {}
        ms_order = []
        for half in range(2):
            for j in range(nb):
                for q in (2 * half, 2 * half + 1):
                    ms_order.append((j, q))
        for idx, (j, q) in enumerate(ms_order):
            ms_at[(j, q)] = idx + 1

        @block.gpsimd
        def _(gpsimd):
            for j, q in ms_order:
                lo_c = woff[2 * q]
                hi_c = woff[2 * q + 1] + span[2 * q + 1]
                gpsimd.memset(Ws[j][:, :, lo_c:hi_c], 0.0).then_inc(ms_sem, 1)
            for i in range(iters):
                j = i % nb
                for m in pool_pairs:
                    gpsimd.wait_ge(dve_sem, 8 * i + m + 1)
                    store_pair(gpsimd, j, m)

        @block.sync
        def _(sync):
            issue_load(sync, 0)
            if iters > 1:
                issue_load(sync, 1)
            for i in range(iters):
                j = i % nb
                for m in sp_pairs:
                    sync.wait_ge(dve_sem, 8 * i + m + 1)
                    store_pair(sync, j, m)
                if i + 2 < iters:
                    # prefetch iter i+2's X: its buffer's last reader is
                    # copies of iter i, all done once dve_sem >= 8(i+1)...
                    # pairs 0-3 done is NOT enough; guarded below by the
                    # scalar-issued guard being unnecessary: loads only
                    # overwrite X[i%nb], whose readers are iter i copies.
                    sync.wait_ge(dve_sem, 8 * (i + 1))
                    issue_load(sync, i + 2)
            sync.wait_ge(dma_sem, dpi * iters)

        @block.scalar
        def _(scalar):
            for i in range(iters):
                j = i % nb
                for m in act_pairs:
                    scalar.wait_ge(dve_sem, 8 * i + m + 1)
                    store_pair(scalar, j, m)

        def _mk_ap(T, offset, dims):
            ap = T[:].copy()
            ap.ap = mybir.VecI64Pair(dims)
            ap.offset = offset
            return ap

        @block.vector
        def _(vector):
            for i in range(iters):
                j = i % nb
                if i >= nb:
                    # W[j]'s data runs about to be overwritten: stores of
                    # iter i-nb must be done.
                    vector.wait_ge(dma_sem, dpi * (i - nb + 1))
                vector.wait_ge(l_sems[j], 16 * (i // nb + 1))
                for m in range(npair):
                    if i < nb and m % 2 == 0:
                        # first use of this buffer: gap zeros must exist
                        vector.wait_ge(ms_sem, ms_at[(j, m // 2)])
                    for jj, (src, dst, ln) in enumerate(runs):
                        # one op covers both lines of the pair (stride s[m]
                        # on the W side, 64 on the X side), both row-groups
                        w_ap = _mk_ap(
                            Ws[j],
                            woff[m] + dst,
                            [[2 * wtot, 128], [wtot, 2], [s[m], 2], [1, ln]],
                        )
                        x_ap = _mk_ap(
                            Xs[j],
                            2 * m * _LW + src,
                            [[2 * _DIN, 128], [_DIN, 2], [_LW, 2], [1, ln]],
                        )
                        ins = vector.tensor_copy(w_ap, x_ap)
                        if jj == len(runs) - 1:
                            ins.then_inc(dve_sem, 1)

    return nc


def _shard_inputs(input_state):
    in_maps = []
    for c in range(_CORES):
        b, q = divmod(c, 4)
        sl = np.ascontiguousarray(
            input_state[b, _RPC * q : _RPC * (q + 1), :], dtype=np.float32
        )
        in_maps.append({"x": sl})
    return in_maps


def _unshard(results, rowmap):
    unmapped = np.setdiff1d(np.arange(_DOUT), rowmap)
    out = np.empty((_B, _DOUT, _DOUT), np.float32)
    for c in range(_CORES):
        b, q = divmod(c, 4)
        shard = results[c]["out"]
        out[b, rowmap[_RPC * q : _RPC * (q + 1)], :] = shard[:_RPC]
        out[b, unmapped[_ZPC * q : _ZPC * (q + 1)], :] = shard[_RPC:]
    return out


def kernel(input_state, P):
    from concourse.bass_utils import run_bass_kernel_spmd

    input_state = np.asarray(input_state)
    rowmap = _rowmap_from_P(P)
    bases, cw, runs = _cluster_structure(rowmap)
    nc = _build_program(bases, cw, runs)
    res = run_bass_kernel_spmd(
        nc, _shard_inputs(input_state), core_ids=list(range(_CORES)), trace=False
    )
    return _unshard(res.results, rowmap)
